# revision 5
# baseline (speedup 1.0000x reference)
"""Top-1 MoE (8 experts) expert-parallel kernel for Trainium2, 8 NeuronCores.

Strategy:
  - Host: argmax(router_logits) -> per-token expert id; tokens for each
    expert are gathered and packed into per-core segments (the "all-to-all
    dispatch" happens host-side since we receive full inputs and return full
    outputs).
  - Load balance: every core runs the same program with nseg fixed-size
    token segments; each segment is bound to one expert's weights (per-core
    choice). A DP bin-covering solver picks segment sizes so per-core
    capacity is close to T/8 regardless of expert imbalance, splitting any
    expert's tokens across cores/segments as needed.
  - Device (SPMD): per segment, dense 2-GEMM SiLU MLP in bf16 with fp32 PSUM
    accumulation. Weight blocks are streamed through SBUF, interleaved
    across segments so DMA stays smooth; within a (block, segment, out-tile)
    group the contraction loop is outer and token chunks inner, so the
    stationary weights are reused across chunks.
  - Host: scatter each segment's outputs back to token order ("combine").

Device layouts (partition-major so every DMA is a plain slice):
  xt  [128, 16, C]        bf16  xt[p, ko, t]     = x[t, ko*128+p]
  w1t [nseg, 128, 16, F]  bf16  w1t[s, p, ko, f] = w1[e(s), f, ko*128+p]
  w2t [nseg, 128, 32, D]  bf16  w2t[s, p, ko, d] = w2[e(s), d, ko*128+p]
  yt  [128, 16, C]        bf16  yt[p, do, t]     = y[t, do*128+p]
"""

import itertools

import numpy as np
import ml_dtypes

BF16 = ml_dtypes.bfloat16

P = 128
D = 2048
F = 4096
E = 8
N_CORES = 8
TCHUNK = 512  # matmul moving free dim (one fp32 PSUM bank)
W1B = 256     # GEMM1 weight block width (columns of F per streamed tile)
W2B = 128     # GEMM2 weight block width (columns of D per streamed tile)

KO1 = D // P  # 16 contraction tiles for GEMM1
KO2 = F // P  # 32 contraction tiles for GEMM2

SEG_OPTS = dict(y_bf16=True, x_kslab=4, wbufs=8, w2_ring="gpsimd")

_BUILD_CACHE = {}


def _seg_chunks(seg_sizes):
    """Per segment: (t0, tw) chunks (<=TCHUNK) in global token coordinates."""
    out = []
    base = 0
    for s in seg_sizes:
        chunks = []
        t0 = 0
        while t0 < s:
            tw = min(TCHUNK, s - t0)
            chunks.append((base + t0, tw))
            t0 += tw
        out.append(chunks)
        base += s
    return out


def build_nc_seg(
    seg_sizes,
    act="silu",
    loop_reps=None,
    w1b=W1B,
    w2b=W2B,
    y_bf16=True,
    x_kslab=4,
    wbufs=8,
    w2_ring="gpsimd",
):
    """Build + compile the per-core Bass program.

    loop_reps wraps one pass in a hardware For_i loop (for slope-based HW
    timing). Results are identical since the computation is idempotent.
    """
    seg_sizes = tuple(int(s) for s in seg_sizes)
    key = ("seg", seg_sizes, act, loop_reps, w1b, w2b, y_bf16, x_kslab, wbufs, w2_ring)
    if key in _BUILD_CACHE:
        return _BUILD_CACHE[key]

    import concourse.bacc as bacc
    import concourse.mybir as mybir
    from concourse import tile

    dt = mybir.dt
    act_fn = {
        "silu": mybir.ActivationFunctionType.Silu,
        "sigmoid": mybir.ActivationFunctionType.Sigmoid,
    }[act]
    nc = bacc.Bacc("TRN2", target_bir_lowering=False, debug=False)

    nseg = len(seg_sizes)
    C = sum(seg_sizes)

    xt_d = nc.dram_tensor("xt", [P, KO1, C], dt.bfloat16, kind="ExternalInput")
    w1t_d = nc.dram_tensor("w1t", [nseg, P, KO1, F], dt.bfloat16, kind="ExternalInput")
    w2t_d = nc.dram_tensor("w2t", [nseg, P, KO2, D], dt.bfloat16, kind="ExternalInput")
    y_dt = dt.bfloat16 if y_bf16 else dt.float32
    yt_d = nc.dram_tensor("yt", [P, KO1, C], y_dt, kind="ExternalOutput")

    chunks_per_seg = _seg_chunks(seg_sizes)
    N1 = F // w1b
    N2 = D // w2b

    with tile.TileContext(nc) as tc:
        with (
            tc.tile_pool(name="xpool", bufs=1) as xpool,
            tc.tile_pool(name="hpool", bufs=1) as hpool,
            tc.tile_pool(name="wpool", bufs=wbufs) as wpool,
            tc.tile_pool(name="w0pool", bufs=2) as w0pool,
            tc.tile_pool(name="ypool", bufs=4) as ypool,
            tc.tile_pool(name="cpool", bufs=1) as cpool,
            tc.tile_pool(name="pspool", bufs=8, space="PSUM") as pspool,
        ):
            zbias = cpool.tile([P, 1], dt.float32)
            nc.any.memset(zbias[:], 0.0)

            x_sb = xpool.tile([P, KO1, C], dt.bfloat16)
            h_sb = hpool.tile([P, KO2, C], dt.bfloat16)

            # x via SWDGE (gpsimd) so it never queues behind the weight
            # stream on the SP HWDGE ring; first chunk split by k-slab so
            # the first matmuls start before the whole chunk lands.
            for si, chunks in enumerate(chunks_per_seg):
                for cj, (t0, tw) in enumerate(chunks):
                    nk = x_kslab if (si == 0 and cj == 0) else 1
                    ks = KO1 // nk
                    for kb in range(nk):
                        nc.gpsimd.dma_start(
                            x_sb[:, kb * ks : (kb + 1) * ks, t0 : t0 + tw],
                            xt_d[:, kb * ks : (kb + 1) * ks, t0 : t0 + tw],
                        )

            def one_pass(rep):
                # GEMM1: h[f, t] = silu(sum_d w1t[d, f] * x[d, t])
                for mb in range(N1):
                    for s in range(nseg):
                        chunks = chunks_per_seg[s]
                        # First block from a dedicated double-buffered pool:
                        # its readers finish early in the pass, so the next
                        # loop iteration's DMA overlaps this pass's GEMM2
                        # instead of stalling the next pass's first matmul.
                        pool = w0pool if (mb == 0 and s == 0) else wpool
                        w1_sb = pool.tile(
                            [P, KO1, w1b], dt.bfloat16, tag="w",
                            name=f"w1_{rep}_{mb}_{s}",
                        )
                        nc.sync.dma_start(
                            w1_sb[:], w1t_d[s, :, :, mb * w1b : (mb + 1) * w1b]
                        )
                        for ms in range(w1b // P):
                            pss = [
                                pspool.tile(
                                    [P, TCHUNK], dt.float32, tag="ps",
                                    name=f"ps1_{rep}_{mb}_{s}_{ms}_{ci}",
                                )
                                for ci in range(len(chunks))
                            ]
                            for k in range(KO1):
                                for ci, (t0, tw) in enumerate(chunks):
                                    nc.tensor.matmul(
                                        pss[ci][:, :tw],
                                        w1_sb[:, k, ms * P : (ms + 1) * P],
                                        x_sb[:, k, t0 : t0 + tw],
                                        start=(k == 0),
                                        stop=(k == KO1 - 1),
                                    )
                            fo = mb * (w1b // P) + ms
                            for ci, (t0, tw) in enumerate(chunks):
                                nc.scalar.activation(
                                    h_sb[:, fo, t0 : t0 + tw],
                                    pss[ci][:, :tw],
                                    act_fn,
                                    bias=zbias[:],
                                )

                # GEMM2: y[d, t] = sum_f w2t[f, d] * h[f, t]
                for db in range(N2):
                    for s in range(nseg):
                        chunks = chunks_per_seg[s]
                        w2_sb = wpool.tile(
                            [P, KO2, w2b], dt.bfloat16, tag="w",
                            name=f"w2_{rep}_{db}_{s}",
                        )
                        w2_eng = getattr(nc, w2_ring) if w2_ring else nc.sync
                        w2_eng.dma_start(
                            w2_sb[:], w2t_d[s, :, :, db * w2b : (db + 1) * w2b]
                        )
                        for ds in range(w2b // P):
                            pss = [
                                pspool.tile(
                                    [P, TCHUNK], dt.float32, tag="ps",
                                    name=f"ps2_{rep}_{db}_{s}_{ds}_{ci}",
                                )
                                for ci in range(len(chunks))
                            ]
                            for k in range(KO2):
                                for ci, (t0, tw) in enumerate(chunks):
                                    nc.tensor.matmul(
                                        pss[ci][:, :tw],
                                        w2_sb[:, k, ds * P : (ds + 1) * P],
                                        h_sb[:, k, t0 : t0 + tw],
                                        start=(k == 0),
                                        stop=(k == KO2 - 1),
                                    )
                            do = db * (w2b // P) + ds
                            for ci, (t0, tw) in enumerate(chunks):
                                y_sb = ypool.tile(
                                    [P, TCHUNK], y_dt, tag="y",
                                    name=f"y_{rep}_{db}_{s}_{ds}_{ci}",
                                )
                                nc.vector.tensor_copy(y_sb[:, :tw], pss[ci][:, :tw])
                                # y stores on the ACT HWDGE ring: never
                                # queues ahead of w1 on the SP ring.
                                nc.scalar.dma_start(
                                    yt_d[:, do, t0 : t0 + tw], y_sb[:, :tw]
                                )

            if loop_reps is not None and loop_reps > 1:
                with tc.For_i(0, loop_reps, 1):
                    one_pass(0)
            else:
                one_pass(0)

    nc.compile()
    _BUILD_CACHE[key] = nc
    return nc


# --- segment-template solver ------------------------------------------------

def solve_assignment(counts, sizes, nbins=8):
    """DP bin covering: can `counts` be covered by `nbins` bins of each size
    in `sizes`? Returns per-expert tuples of per-size bin counts, or None."""
    nseg = len(sizes)

    def covers(c):
        opts = []
        for ks in itertools.product(*(range(nbins + 1) for _ in range(nseg))):
            cap = sum(k * s for k, s in zip(ks, sizes))
            if cap < c:
                continue
            ok = True
            for j in range(nseg):
                if ks[j] > 0 and cap - sizes[j] >= c:
                    ok = False
                    break
            if ok:
                opts.append(ks)
        return opts

    levels = [{(0,) * nseg: None}]
    for c in counts:
        opts = covers(c)
        new = {}
        for state in levels[-1]:
            for ks in opts:
                ns = tuple(u + k for u, k in zip(state, ks))
                if all(u <= nbins for u in ns) and ns not in new:
                    new[ns] = (state, ks)
        if not new:
            return None
        levels.append(new)
    state = next(iter(levels[-1]))
    alloc = []
    for lev in range(len(counts), 0, -1):
        prev, ks = levels[lev][state]
        alloc.append(ks)
        state = prev
    return alloc[::-1]


def _chunk_count(size):
    return -(-size // TCHUNK)


def seg_cost(sizes):
    """Estimated per-core ns: PE column streaming + per-MM overhead + per-
    segment weight-stream overhead. Constants fit on HW (2026-08)."""
    C = sum(sizes)
    n_mm = sum(_chunk_count(s) for s in sizes) * 1024
    return C * 1024 / 2.4 + n_mm * 12.0 + len(sizes) * 3000.0


def solve_segments(counts, nbins=8):
    """Enumerate segment-size templates, DP-check feasibility, minimize
    seg_cost. Returns (sizes, alloc)."""
    best = None

    def consider(sizes):
        nonlocal best
        cost = seg_cost(sizes)
        if best is not None and cost >= best[0]:
            return False
        alloc = solve_assignment(counts, sizes, nbins)
        if alloc is not None:
            best = (cost, tuple(sizes), alloc)
            return True
        return False

    cmax = max(P, max(counts))
    consider((cmax,))
    # 2seg
    for S1 in range(512, cmax + 64, 64):
        for S2 in range(4, min(S1, 520) + 1, 4):
            if best is not None and seg_cost((S1, S2)) >= best[0]:
                continue
            consider((S1, S2))
    # 3seg
    for S1 in range(512, cmax + 64, 64):
        for S2 in range(16, min(S1, 520) + 1, 16):
            for S3 in range(4, S2 + 1, 4):
                if best is not None and seg_cost((S1, S2, S3)) >= best[0]:
                    continue
                consider((S1, S2, S3))
    return best[1], best[2]


# --- host packing -----------------------------------------------------------

def _pack_w1(w1_e):
    """w1_e [F, D] f32 -> [128, KO1, F] bf16."""
    return np.ascontiguousarray(
        w1_e.astype(BF16).reshape(F, KO1, P).transpose(2, 1, 0)
    )


def _pack_w2(w2_e):
    """w2_e [D, F] f32 -> [128, KO2, D] bf16."""
    return np.ascontiguousarray(
        w2_e.astype(BF16).reshape(D, KO2, P).transpose(2, 1, 0)
    )


LAST_RUN = {}


def prepare(hidden_states, router_logits, w1, w2):
    """Host-side routing + balanced segment packing.

    Returns (nc, in_maps, meta)."""
    hidden_states = np.asarray(hidden_states)
    router_logits = np.asarray(router_logits)
    w1 = np.asarray(w1)
    w2 = np.asarray(w2)

    b, s, d = hidden_states.shape
    T = b * s
    x = hidden_states.reshape(T, d).astype(np.float32)
    assign = np.argmax(router_logits.reshape(T, E), axis=-1)

    idx = [np.nonzero(assign == e)[0] for e in range(E)]
    counts = [int(i.size) for i in idx]

    try:
        sizes, alloc = solve_segments(counts)
    except Exception:
        sizes = (max(P, max(counts)),)
        alloc = [(1,) for _ in counts]
    nseg = len(sizes)
    C = sum(sizes)
    nc = build_nc_seg(sizes, **SEG_OPTS)

    # bins[j] = per segment slot j: list of (expert, token_idx_array).
    bins = [[] for _ in range(nseg)]
    for e in range(E):
        pos = 0
        for j in range(nseg):
            for _ in range(alloc[e][j]):
                take = min(sizes[j], counts[e] - pos)
                bins[j].append((e, idx[e][pos : pos + take]))
                pos += take
        assert pos == counts[e], (e, pos, counts[e])
    for j in range(nseg):
        while len(bins[j]) < N_CORES:
            bins[j].append((0, np.zeros(0, dtype=np.int64)))
        assert len(bins[j]) == N_CORES

    w1_packed, w2_packed = {}, {}

    def packed(e):
        if e not in w1_packed:
            w1_packed[e] = _pack_w1(w1[e])
            w2_packed[e] = _pack_w2(w2[e])
        return w1_packed[e], w2_packed[e]

    seg_starts = np.cumsum([0] + list(sizes))[:-1]
    in_maps = []
    core_bins = []
    for c in range(N_CORES):
        xb = np.zeros((C, D), dtype=BF16)
        experts = []
        idxs = []
        for j in range(nseg):
            e, tok = bins[j][c]
            xb[seg_starts[j] : seg_starts[j] + len(tok)] = x[tok].astype(BF16)
            experts.append(e)
            idxs.append(tok)
        xt = np.ascontiguousarray(xb.reshape(C, KO1, P).transpose(2, 1, 0))
        p1 = np.stack([packed(e)[0] for e in experts])
        p2 = np.stack([packed(e)[1] for e in experts])
        in_maps.append(
            {
                "xt": xt,
                "w1t": np.ascontiguousarray(p1),
                "w2t": np.ascontiguousarray(p2),
            }
        )
        core_bins.append(idxs)

    meta = {
        "mode": "seg", "b": b, "s": s, "d": d, "T": T, "C": C,
        "sizes": sizes, "seg_starts": seg_starts, "core_bins": core_bins,
        "counts": counts,
    }
    return nc, in_maps, meta


def finish(results, meta):
    """Scatter per-core segment outputs back to token order."""
    T, d, C = meta["T"], meta["d"], meta["C"]
    seg_starts = meta["seg_starts"]
    out = np.zeros((T, d), dtype=np.float32)
    for c in range(N_CORES):
        yt = np.asarray(results[c]["yt"])  # [128, KO1, C]
        y_tok = yt.transpose(2, 1, 0).reshape(C, D).astype(np.float32)
        for j, tok in enumerate(meta["core_bins"][c]):
            if len(tok):
                out[tok] = y_tok[seg_starts[j] : seg_starts[j] + len(tok)]
    return out.reshape(meta["b"], meta["s"], d)


def kernel(hidden_states, router_logits, w1, w2):
    from concourse.bass_utils import run_bass_kernel_spmd

    nc, in_maps, meta = prepare(hidden_states, router_logits, w1, w2)
    res = run_bass_kernel_spmd(nc, in_maps, core_ids=list(range(N_CORES)))
    LAST_RUN["capacity"] = meta["C"]
    LAST_RUN["counts"] = meta["counts"]
    LAST_RUN["sizes"] = meta["sizes"]
    return finish(res.results, meta)
